# revision 1
# baseline (speedup 1.0000x reference)
"""Trainium2 Bass kernel for DYSPN-style dynamic local filtering.

Computation (per batch b, pixel p):
    patches[j,p] = 7x7 im2col of `input` (zero pad 3), center tap replaced by input0
    scale[j,p]   = attention[b, i, ring(j), p]      (ring in {0..3}, scale >= 0)
    w            = kernel * scale;  w /= sum_j |w|
    out[p]       = sum_j patches[j,p] * w[j,p]

Since scale >= 0 and constant within a ring (ring = Chebyshev distance from
the center tap):
    out = (sum_r att_r * B_r) / (sum_r att_r * A_r)
    B_r = sum_{j in ring r} patches_j * k_j,   A_r = sum_{j in ring r} |k_j|

Sharding: 8 cores = 4 batches x 2 half-images (128 rows each). Per core the
output plane is [128 rows (partitions), 320 cols (free)]; tap shifts become
free-dim offsets into 7 pre-shifted padded-image variants (host-built).
Each ring's taps form a regular (dy x dx-step) lattice, so all 49 tap
multiplies collapse into 10 DVE ops via multi-dim overlapping views of the
shifted-image tile. Ring sums are pairwise tensor_add trees (tensor_tensor
streams 1 output/cycle using both read ports; strided tensor_reduce only
manages ~0.6/cycle). |k| runs on ScalarE, which has its own SBUF port; the
|k| planes live 48 planes above the k planes in one tile so each B-tree op
also carries the matching A-tree level as a second AP dim. GpSimd is left
idle on purpose: its SBUF port is DVE's second read port, so any concurrent
POOL op halves DVE throughput.
"""

import sys

for _p in ("/opt/trn_rl_repo", "/root/.axon_site"):
    if _p not in sys.path:
        sys.path.insert(0, _p)

import numpy as np
from contextlib import ExitStack

import concourse.bass as bass
import concourse.tile as tile
from concourse import bacc, mybir
from concourse.bass_utils import run_bass_kernel_spmd

H, W = 256, 320
BS = 4
KK = 49
HALF_ROWS = 128
PAD_W = W + 6  # 326
APLANE = 48  # |k| plane j lives at kall plane j + 48 (j = 1..48)

def _ring_ids() -> np.ndarray:
    ring = np.zeros(KK, dtype=np.int32)
    for j in range(KK):
        dy, dx = divmod(j, 7)
        ring[j] = max(abs(dy - 3), abs(dx - 3))
    return ring

_RING = _ring_ids()
RING_TAPS = [np.where(_RING == r)[0].tolist() for r in range(4)]  # 1,8,16,24
RING_ORDER = np.concatenate([np.asarray(t) for t in RING_TAPS]).astype(np.int64)

# plane ranges of each ring inside the [128, 49, 320] ring-ordered k region
RING_OFF = [0, 1, 9, 25, 49]

def _mul_ops(r):
    """Tap-multiply op shapes for ring r>=1: (rel_plane, n_planes, img_dims, img_off).

    Ring taps in j-order: top row (2r+1), middle 2r-1 rows with dx in
    {3-r, 3+r}, bottom row (2r+1). img_dims are AP dims [stride, num]
    prepended to [1, W]; img_off indexes the [7, 326] shifted-image block.
    """
    n = 2 * r + 1
    lo = 3 - r
    return [
        (0, n, [[1, n]], lo * PAD_W + lo),
        (n, 2 * (n - 2), [[PAD_W, n - 2], [2 * r, 2]], (lo + 1) * PAD_W + lo),
        (n + 2 * (n - 2), n, [[1, n]], (lo + n - 1) * PAD_W + lo),
    ]

_NC = None
LAST_RESULTS = None


def _build_program():
    f32 = mybir.dt.float32
    nc = bacc.Bacc("TRN2", target_bir_lowering=False, debug=False, num_devices=8)
    k_d = nc.dram_tensor("k", [HALF_ROWS, KK, W], f32, kind="ExternalInput").ap()
    img7_d = nc.dram_tensor("img7", [HALF_ROWS, 7, PAD_W], f32, kind="ExternalInput").ap()
    in0_d = nc.dram_tensor("in0", [HALF_ROWS, W], f32, kind="ExternalInput").ap()
    att_d = nc.dram_tensor("att", [HALF_ROWS, 8, W], f32, kind="ExternalInput").ap()
    out_d = nc.dram_tensor("out", [HALF_ROWS, W], f32, kind="ExternalOutput").ap()

    with tile.TileContext(nc) as tc, ExitStack() as ctx:
        pool = ctx.enter_context(tc.tile_pool(name="main", bufs=1))

        # planes 0:49 = ring-ordered k (rings 2-3 in-place become patches*k);
        # planes 49:57 = ring1 patches*k (kept separate so ring1's multiplies
        # don't wait on the |k| pass in the latency-critical early window);
        # planes 57:97 = |k| rings 2-3; planes 97:105 = |k| ring1
        kall = pool.tile([HALF_ROWS, 105, W], f32, name="kall")
        img7_t = pool.tile([HALF_ROWS, 7, PAD_W], f32)
        in0_t = pool.tile([HALF_ROWS, W], f32)
        att_t = pool.tile([HALF_ROWS, 8, W], f32)  # att duplicated for B|A paths
        # planes 0:4 = B_r, planes 4:8 = A_r
        res = pool.tile([HALF_ROWS, 8, W], f32)

        kall_ap = kall[:]
        kpart = kall_ap.ap[0]
        img7_ap = img7_t[:]
        ipart = img7_ap.ap[0]

        def kap(plane, dims):
            return bass.AP(kall_ap.tensor, kall_ap.offset + plane * W,
                           [kpart] + dims)

        def iap(off, dims):
            return bass.AP(img7_ap.tensor, img7_ap.offset + off,
                           [ipart] + dims + [[1, W]])

        # ---- DMAs, ordered by when compute needs the data
        nc.sync.dma_start(kall[:, 1:5, :], k_d[:, 1:5, :])          # ring1
        nc.sync.dma_start(img7_t[:, 2, :], img7_d[:, 2, :])
        nc.sync.dma_start(in0_t[:], in0_d[:])
        nc.sync.dma_start(kall[:, 0:1, :], k_d[:, 0:1, :])          # center
        nc.sync.dma_start(kall[:, 5:9, :], k_d[:, 5:9, :])
        for t in (3, 4):                                            # img rows for ring1
            nc.sync.dma_start(img7_t[:, t, :], img7_d[:, t, :])
        nc.sync.dma_start(kall[:, 9:17, :], k_d[:, 9:17, :])        # ring2
        nc.sync.dma_start(kall[:, 17:25, :], k_d[:, 17:25, :])
        for t in (1, 5):
            nc.sync.dma_start(img7_t[:, t, :], img7_d[:, t, :])
        nc.sync.dma_start(kall[:, 25:33, :], k_d[:, 25:33, :])      # ring3
        for t in (0, 6):
            nc.sync.dma_start(img7_t[:, t, :], img7_d[:, t, :])
        nc.sync.dma_start(kall[:, 33:41, :], k_d[:, 33:41, :])
        nc.sync.dma_start(kall[:, 41:49, :], k_d[:, 41:49, :])
        nc.sync.dma_start(att_t[:], att_d[:])

        # ---- |k| on ScalarE (rings 2-3 must read k before the in-place
        # multiplies; ring1 writes elsewhere so there is no ordering)
        Abs = mybir.ActivationFunctionType.Abs
        nc.scalar.activation(kall[:, 97:101, :], kall[:, 1:5, :], Abs)  # ring1
        nc.scalar.activation(kall[:, 101:105, :], kall[:, 5:9, :], Abs)
        nc.scalar.activation(res[:, 4, :], kall[:, 0, :], Abs)          # A_0
        for o in range(9, KK, 8):
            nc.scalar.activation(kall[:, o + APLANE:o + 8 + APLANE, :],
                                 kall[:, o:o + 8, :], Abs)

        # ---- patches*k multiplies (3 fused ops per ring + center);
        # ring1 products land at planes 49:57, rings 2-3 in place
        first = True
        for r in (1, 2, 3):
            for (rel, n_pl, img_dims, img_off) in _mul_ops(r):
                o = RING_OFF[r] + rel
                oo = o + APLANE if r == 1 else o
                nc.vector.tensor_mul(kall[:, oo:oo + n_pl, :],
                                     kall[:, o:o + n_pl, :],
                                     iap(img_off, img_dims))
                if first:
                    # center tap fills the wait for the second ring1 chunk
                    nc.vector.tensor_mul(res[:, 0, :], kall[:, 0, :], in0_t[:])
                    first = False

        # ---- ring sums: each op handles the B level and the A level (48
        # planes up) through a paired leading AP dim
        def paired_fold(base, h, delta):
            """kall[{base, base+48}][0:h] += kall[{base+delta, ...}][0:h]"""
            dims = [[APLANE * W, 2], [W, h], [1, W]]
            nc.vector.tensor_add(kap(base, dims), kap(base, dims),
                                 kap(base + delta, dims))

        def paired_tree(base, sz, r):
            cur = sz
            while cur > 2:
                paired_fold(base, cur // 2, cur // 2)
                cur //= 2
            dims = [[APLANE * W, 2], [1, W]]
            rdims = [[4 * W, 2], [1, W]]
            nc.vector.tensor_add(
                bass.AP(res[:].tensor, res[:].offset + r * W, [res[:].ap[0]] + rdims),
                kap(base, dims), kap(base + 1, dims))

        paired_tree(49, 8, 1)                    # ring1 (pk 49:57, |k| 97:105)
        paired_tree(9, 16, 2)                    # ring2
        paired_fold(25, 8, 8)                    # ring3: fold chunks
        paired_fold(25, 8, 16)
        paired_tree(25, 8, 3)

        # ---- combine + divide
        pnd = pool.tile([HALF_ROWS, 8, W], f32)
        nc.vector.tensor_mul(pnd[:], att_t[:], res[:])
        pnd_ap = pnd[:]
        ppart = pnd_ap.ap[0]

        def pap(plane, dims):
            return bass.AP(pnd_ap.tensor, pnd_ap.offset + plane * W,
                           [ppart] + dims)

        d2 = [[4 * W, 2], [W, 2], [1, W]]
        nc.vector.tensor_add(pap(0, d2), pap(0, d2), pap(2, d2))
        d1 = [[4 * W, 2], [1, W]]
        nc.vector.tensor_add(pap(0, d1), pap(0, d1), pap(1, d1))

        rden_t = pool.tile([HALF_ROWS, W], f32)
        scr_t = pool.tile([HALF_ROWS, W], f32)
        nc.vector.reciprocal_approx_accurate(rden_t[:], pnd[:, 4, :], scr_t[:])
        out_t = pool.tile([HALF_ROWS, W], f32)
        nc.vector.tensor_mul(out_t[:], pnd[:, 0, :], rden_t[:])
        nc.sync.dma_start(out_d[:], out_t[:])

    nc.compile()
    return nc


def _get_program():
    global _NC
    if _NC is None:
        _NC = _build_program()
    return _NC


def kernel(**inputs) -> np.ndarray:
    k = np.asarray(inputs["kernel"], dtype=np.float32)      # [4, 49, 81920]
    img = np.asarray(inputs["input"], dtype=np.float32)     # [4, 1, 256, 320]
    in0 = np.asarray(inputs["input0"], dtype=np.float32)    # [4, 1, 256, 320]
    att = np.asarray(inputs["attention"], dtype=np.float32) # [4, 6, 4, 81920]
    ii = int(np.asarray(inputs["i"]))

    nc = _get_program()

    in_maps = []
    for c in range(8):
        b, half = divmod(c, 2)
        y0 = half * HALF_ROWS
        kc = k[b][RING_ORDER][:, y0 * W:(y0 + HALF_ROWS) * W]
        kc = np.ascontiguousarray(kc.reshape(KK, HALF_ROWS, W).transpose(1, 0, 2))
        pad = np.zeros((HALF_ROWS + 6, PAD_W), np.float32)
        lo, hi = max(0, y0 - 3), min(H, y0 + HALF_ROWS + 3)
        pad[lo - (y0 - 3):hi - (y0 - 3), 3:3 + W] = img[b, 0, lo:hi]
        img7 = np.ascontiguousarray(
            np.stack([pad[t:t + HALF_ROWS] for t in range(7)], axis=1))
        in0c = np.ascontiguousarray(in0[b, 0, y0:y0 + HALF_ROWS])
        attc = att[b, ii][:, y0 * W:(y0 + HALF_ROWS) * W]
        attc = attc.reshape(4, HALF_ROWS, W).transpose(1, 0, 2)
        att2 = np.ascontiguousarray(np.concatenate([attc, attc], axis=1))
        in_maps.append({"k": kc, "img7": img7, "in0": in0c, "att": att2})

    res = run_bass_kernel_spmd(nc, in_maps, list(range(8)))
    global LAST_RESULTS
    LAST_RESULTS = res

    out = np.empty((BS, 1, H, W), np.float32)
    for c in range(8):
        b, half = divmod(c, 2)
        out[b, 0, half * HALF_ROWS:(half + 1) * HALF_ROWS] = res.results[c]["out"]
    return out



# revision 2
# speedup vs baseline: 1.2495x; 1.2495x over previous
"""Trainium2 Bass kernel for DYSPN-style dynamic local filtering.

Computation (per batch b, pixel p):
    patches[j,p] = 7x7 im2col of `input` (zero pad 3), center tap replaced by input0
    scale[j,p]   = attention[b, i, ring(j), p]      (ring in {0..3}, scale >= 0)
    w            = kernel * scale;  w /= sum_j |w|
    out[p]       = sum_j patches[j,p] * w[j,p]

Since scale >= 0 and constant within a ring (ring = Chebyshev distance from
the center tap):
    out = (sum_r att_r * B_r) / (sum_r att_r * A_r)
    B_r = sum_{j in ring r} patches_j * k_j,   A_r = sum_{j in ring r} |k_j|

Sharding: 8 cores = 4 batches x 2 half-images (128 rows each). Per core the
output plane is [128 rows (partitions), 320 cols (free)]; tap shifts become
free-dim offsets into 7 pre-shifted padded-image variants (host-built).
Each ring's taps form a regular (dy x dx-step) lattice, so all 49 tap
multiplies collapse into 10 DVE ops via multi-dim overlapping views of the
shifted-image tile. Ring sums are pairwise tensor_add trees (tensor_tensor
streams 1 output/cycle using both read ports; strided tensor_reduce only
manages ~0.6/cycle). |k| runs on ScalarE, which has its own SBUF port; the
|k| planes live 48 planes above the k planes in one tile so each B-tree op
also carries the matching A-tree level as a second AP dim. GpSimd is left
idle on purpose: its SBUF port is DVE's second read port, so any concurrent
POOL op halves DVE throughput.

All streaming data is fp16: the DVE 2x/4x perf modes require 2-byte packed
operands, and DMA bytes halve. Only the final divide runs in fp32 (the
reciprocal custom op depends on fp32 bit layout).
"""

import sys

for _p in ("/opt/trn_rl_repo", "/root/.axon_site"):
    if _p not in sys.path:
        sys.path.insert(0, _p)

import numpy as np
from contextlib import ExitStack

import concourse.bass as bass
import concourse.tile as tile
from concourse import bacc, mybir
from concourse.bass_utils import run_bass_kernel_spmd

H, W = 256, 320
BS = 4
KK = 49
HALF_ROWS = 128
PAD_W = W + 6  # 326
APLANE = 48  # |k| plane j lives at kall plane j + 48 (j = 1..48)

def _ring_ids() -> np.ndarray:
    ring = np.zeros(KK, dtype=np.int32)
    for j in range(KK):
        dy, dx = divmod(j, 7)
        ring[j] = max(abs(dy - 3), abs(dx - 3))
    return ring

_RING = _ring_ids()
RING_TAPS = [np.where(_RING == r)[0].tolist() for r in range(4)]  # 1,8,16,24
RING_ORDER = np.concatenate([np.asarray(t) for t in RING_TAPS]).astype(np.int64)

# plane ranges of each ring inside the [128, 49, 320] ring-ordered k region
RING_OFF = [0, 1, 9, 25, 49]

def _mul_ops(r):
    """Tap-multiply op shapes for ring r>=1: (rel_plane, n_planes, img_dims, img_off).

    Ring taps in j-order: top row (2r+1), middle 2r-1 rows with dx in
    {3-r, 3+r}, bottom row (2r+1). img_dims are AP dims [stride, num]
    prepended to [1, W]; img_off indexes the [7, 326] shifted-image block.
    """
    n = 2 * r + 1
    lo = 3 - r
    return [
        (0, n, [[1, n]], lo * PAD_W + lo),
        (n, 2 * (n - 2), [[PAD_W, n - 2], [2 * r, 2]], (lo + 1) * PAD_W + lo),
        (n + 2 * (n - 2), n, [[1, n]], (lo + n - 1) * PAD_W + lo),
    ]

_NC = None
LAST_RESULTS = None


def _build_program():
    f16 = mybir.dt.float16
    f32 = mybir.dt.float32
    nc = bacc.Bacc("TRN2", target_bir_lowering=False, debug=False, num_devices=8)
    k_d = nc.dram_tensor("k", [HALF_ROWS, KK, W], f16, kind="ExternalInput").ap()
    img7_d = nc.dram_tensor("img7", [HALF_ROWS, 7, PAD_W], f16, kind="ExternalInput").ap()
    in0_d = nc.dram_tensor("in0", [HALF_ROWS, W], f16, kind="ExternalInput").ap()
    att_d = nc.dram_tensor("att", [HALF_ROWS, 4, W], f16, kind="ExternalInput").ap()
    out_d = nc.dram_tensor("out", [HALF_ROWS, W], f32, kind="ExternalOutput").ap()

    with tile.TileContext(nc) as tc, ExitStack() as ctx:
        pool = ctx.enter_context(tc.tile_pool(name="main", bufs=1))

        # planes 0:49 = ring-ordered k (rings 2-3 in-place become patches*k);
        # planes 49:57 = ring1 patches*k (kept separate so ring1's multiplies
        # don't wait on the |k| pass in the latency-critical early window);
        # planes 57:97 = |k| rings 2-3; planes 97:105 = |k| ring1
        kall = pool.tile([HALF_ROWS, 105, W], f16, name="kall")
        img7_t = pool.tile([HALF_ROWS, 7, PAD_W], f16)
        in0_t = pool.tile([HALF_ROWS, W], f16)
        att_t = pool.tile([HALF_ROWS, 4, W], f16)
        # planes 0:4 = B_r, planes 4:8 = A_r
        res = pool.tile([HALF_ROWS, 8, W], f16)

        kall_ap = kall[:]
        kpart = kall_ap.ap[0]
        img7_ap = img7_t[:]
        ipart = img7_ap.ap[0]

        def kap(plane, dims):
            return bass.AP(kall_ap.tensor, kall_ap.offset + plane * W,
                           [kpart] + dims)

        def iap(off, dims):
            return bass.AP(img7_ap.tensor, img7_ap.offset + off,
                           [ipart] + dims + [[1, W]])

        # ---- DMAs, ordered by when compute needs the data
        nc.sync.dma_start(kall[:, 1:5, :], k_d[:, 1:5, :])          # ring1
        nc.sync.dma_start(img7_t[:, 2, :], img7_d[:, 2, :])
        nc.sync.dma_start(in0_t[:], in0_d[:])
        nc.sync.dma_start(kall[:, 0:1, :], k_d[:, 0:1, :])          # center
        nc.sync.dma_start(kall[:, 5:9, :], k_d[:, 5:9, :])
        for t in (3, 4):                                            # img rows for ring1
            nc.sync.dma_start(img7_t[:, t, :], img7_d[:, t, :])
        nc.sync.dma_start(kall[:, 9:17, :], k_d[:, 9:17, :])        # ring2
        nc.sync.dma_start(kall[:, 17:25, :], k_d[:, 17:25, :])
        for t in (1, 5):
            nc.sync.dma_start(img7_t[:, t, :], img7_d[:, t, :])
        nc.sync.dma_start(kall[:, 25:33, :], k_d[:, 25:33, :])      # ring3
        for t in (0, 6):
            nc.sync.dma_start(img7_t[:, t, :], img7_d[:, t, :])
        nc.sync.dma_start(kall[:, 33:41, :], k_d[:, 33:41, :])
        nc.sync.dma_start(kall[:, 41:49, :], k_d[:, 41:49, :])
        nc.sync.dma_start(att_t[:], att_d[:])

        # ---- |k| on ScalarE (rings 2-3 must read k before the in-place
        # multiplies; ring1 writes elsewhere so there is no ordering)
        Abs = mybir.ActivationFunctionType.Abs
        nc.scalar.activation(kall[:, 97:101, :], kall[:, 1:5, :], Abs)  # ring1
        nc.scalar.activation(kall[:, 101:105, :], kall[:, 5:9, :], Abs)
        nc.scalar.activation(res[:, 4, :], kall[:, 0, :], Abs)          # A_0
        for o in range(9, KK, 8):
            nc.scalar.activation(kall[:, o + APLANE:o + 8 + APLANE, :],
                                 kall[:, o:o + 8, :], Abs)

        # ---- patches*k multiplies (3 fused ops per ring + center);
        # ring1 products land at planes 49:57, rings 2-3 in place
        first = True
        for r in (1, 2, 3):
            for (rel, n_pl, img_dims, img_off) in _mul_ops(r):
                o = RING_OFF[r] + rel
                oo = o + APLANE if r == 1 else o
                nc.vector.tensor_mul(kall[:, oo:oo + n_pl, :],
                                     kall[:, o:o + n_pl, :],
                                     iap(img_off, img_dims))
                if first:
                    # center tap fills the wait for the second ring1 chunk
                    nc.vector.tensor_mul(res[:, 0, :], kall[:, 0, :], in0_t[:])
                    first = False

        # ---- ring sums: each op handles the B level and the A level (48
        # planes up) through a paired leading AP dim
        def paired_fold(base, h, delta):
            """kall[{base, base+48}][0:h] += kall[{base+delta, ...}][0:h]"""
            dims = [[APLANE * W, 2], [W, h], [1, W]]
            nc.vector.tensor_add(kap(base, dims), kap(base, dims),
                                 kap(base + delta, dims))

        def paired_tree(base, sz, r):
            cur = sz
            while cur > 2:
                paired_fold(base, cur // 2, cur // 2)
                cur //= 2
            dims = [[APLANE * W, 2], [1, W]]
            rdims = [[4 * W, 2], [1, W]]
            nc.vector.tensor_add(
                bass.AP(res[:].tensor, res[:].offset + r * W, [res[:].ap[0]] + rdims),
                kap(base, dims), kap(base + 1, dims))

        paired_tree(49, 8, 1)                    # ring1 (pk 49:57, |k| 97:105)
        paired_tree(9, 16, 2)                    # ring2
        paired_fold(25, 8, 8)                    # ring3: fold chunks
        paired_fold(25, 8, 16)
        paired_tree(25, 8, 3)

        # ---- combine + divide (att read twice: B planes, then A planes)
        pnd = pool.tile([HALF_ROWS, 8, W], f16)
        nc.vector.tensor_mul(pnd[:, 0:4, :], att_t[:], res[:, 0:4, :])
        nc.vector.tensor_mul(pnd[:, 4:8, :], att_t[:], res[:, 4:8, :])
        pnd_ap = pnd[:]
        ppart = pnd_ap.ap[0]

        def pap(plane, dims):
            return bass.AP(pnd_ap.tensor, pnd_ap.offset + plane * W,
                           [ppart] + dims)

        d2 = [[4 * W, 2], [W, 2], [1, W]]
        nc.vector.tensor_add(pap(0, d2), pap(0, d2), pap(2, d2))
        d1 = [[4 * W, 2], [1, W]]
        nc.vector.tensor_add(pap(0, d1), pap(0, d1), pap(1, d1))

        # divide in fp32: reciprocal_approx depends on fp32 bit layout
        den_t = pool.tile([HALF_ROWS, W], f32)
        nc.scalar.copy(den_t[:], pnd[:, 4, :])
        rden_t = pool.tile([HALF_ROWS, W], f32)
        scr_t = pool.tile([HALF_ROWS, W], f32)
        nc.vector.reciprocal_approx_accurate(rden_t[:], den_t[:], scr_t[:])
        out_t = pool.tile([HALF_ROWS, W], f32)
        nc.vector.tensor_mul(out_t[:], pnd[:, 0, :], rden_t[:])
        nc.sync.dma_start(out_d[:], out_t[:])

    nc.compile()
    return nc


def _get_program():
    global _NC
    if _NC is None:
        _NC = _build_program()
    return _NC


def kernel(**inputs) -> np.ndarray:
    k = np.asarray(inputs["kernel"], dtype=np.float32)      # [4, 49, 81920]
    img = np.asarray(inputs["input"], dtype=np.float32)     # [4, 1, 256, 320]
    in0 = np.asarray(inputs["input0"], dtype=np.float32)    # [4, 1, 256, 320]
    att = np.asarray(inputs["attention"], dtype=np.float32) # [4, 6, 4, 81920]
    ii = int(np.asarray(inputs["i"]))

    k16 = k.astype(np.float16)
    img16 = img.astype(np.float16)
    in016 = in0.astype(np.float16)
    att16 = att[:, ii].astype(np.float16)                   # [4, 4, 81920]

    nc = _get_program()

    in_maps = []
    for c in range(8):
        b, half = divmod(c, 2)
        y0 = half * HALF_ROWS
        kc = k16[b][RING_ORDER][:, y0 * W:(y0 + HALF_ROWS) * W]
        kc = np.ascontiguousarray(kc.reshape(KK, HALF_ROWS, W).transpose(1, 0, 2))
        pad = np.zeros((HALF_ROWS + 6, PAD_W), np.float16)
        lo, hi = max(0, y0 - 3), min(H, y0 + HALF_ROWS + 3)
        pad[lo - (y0 - 3):hi - (y0 - 3), 3:3 + W] = img16[b, 0, lo:hi]
        img7 = np.ascontiguousarray(
            np.stack([pad[t:t + HALF_ROWS] for t in range(7)], axis=1))
        in0c = np.ascontiguousarray(in016[b, 0, y0:y0 + HALF_ROWS])
        attc = att16[b][:, y0 * W:(y0 + HALF_ROWS) * W]
        attc = np.ascontiguousarray(
            attc.reshape(4, HALF_ROWS, W).transpose(1, 0, 2))
        in_maps.append({"k": kc, "img7": img7, "in0": in0c, "att": attc})

    res = run_bass_kernel_spmd(nc, in_maps, list(range(8)))
    global LAST_RESULTS
    LAST_RESULTS = res

    out = np.empty((BS, 1, H, W), np.float32)
    for c in range(8):
        b, half = divmod(c, 2)
        out[b, 0, half * HALF_ROWS:(half + 1) * HALF_ROWS] = res.results[c]["out"]
    return out


# revision 3
# speedup vs baseline: 1.5139x; 1.2115x over previous
"""Trainium2 Bass kernel for DYSPN-style dynamic local filtering.

Computation (per batch b, pixel p):
    patches[j,p] = 7x7 im2col of `input` (zero pad 3), center tap replaced by input0
    scale[j,p]   = attention[b, i, ring(j), p]      (ring in {0..3}, scale >= 0)
    w            = kernel * scale;  w /= sum_j |w|
    out[p]       = sum_j patches[j,p] * w[j,p]

Since scale >= 0 and constant within a ring (ring = Chebyshev distance from
the center tap):
    out = (sum_r att_r * B_r) / (sum_r att_r * A_r)
    B_r = sum_{j in ring r} patches_j * k_j,   A_r = sum_{j in ring r} |k_j|

Sharding: 8 cores = 4 batches x 2 half-images (128 rows each). Per core the
output plane is [128 rows (partitions), 320 cols (free)]; tap shifts become
free-dim offsets into 7 pre-shifted padded-image variants (host-built).
Each ring's taps form a regular (dy x dx-step) lattice, so all 49 tap
multiplies collapse into 10 DVE ops via multi-dim overlapping views of the
shifted-image tile. Ring sums are pairwise tensor_add trees (tensor_tensor
streams ~1.6 elem/cycle fp16 using both read ports; strided tensor_reduce is
far slower and has no fp16 fast mode). |k| runs on ScalarE, which has its
own SBUF port; products land in separate planes (not in-place over k) so the
scalar |k| pass never gates the DVE multiplies. The |k| plane for tap j sits
48 planes above its product plane, so each B-tree op also carries the
matching A-tree level through a paired leading AP dim. GpSimd is left idle:
its SBUF port is DVE's second read port, and its sw ALU is ~2ns/elem anyway.

All streaming data is fp16 (DVE 2x perf mode needs 2-byte packed operands;
DMA bytes halve). Only the final divide runs in fp32 (the reciprocal custom
op depends on fp32 bit layout).
"""

import sys

for _p in ("/opt/trn_rl_repo", "/root/.axon_site"):
    if _p not in sys.path:
        sys.path.insert(0, _p)

import numpy as np
from contextlib import ExitStack

import concourse.bass as bass
import concourse.tile as tile
from concourse import bacc, mybir
from concourse.bass_utils import run_bass_kernel_spmd

H, W = 256, 320
BS = 4
KK = 49
HALF_ROWS = 128
PAD_W = W + 6  # 326
PBASE = 49   # product plane for tap j = PBASE + j
ABASE = 97   # |k| plane for tap j = ABASE + j  (j = 1..48)
APLANE = ABASE - PBASE  # = 48: A tree runs 48 planes above the P tree

def _ring_ids() -> np.ndarray:
    ring = np.zeros(KK, dtype=np.int32)
    for j in range(KK):
        dy, dx = divmod(j, 7)
        ring[j] = max(abs(dy - 3), abs(dx - 3))
    return ring

_RING = _ring_ids()
RING_TAPS = [np.where(_RING == r)[0].tolist() for r in range(4)]  # 1,8,16,24
RING_ORDER = np.concatenate([np.asarray(t) for t in RING_TAPS]).astype(np.int64)

# plane ranges of each ring inside the ring-ordered k region
RING_OFF = [0, 1, 9, 25, 49]

def _mul_ops(r):
    """Tap-multiply op shapes for ring r>=1: (rel_plane, n_planes, img_dims, img_off).

    Ring taps in j-order: top row (2r+1), middle 2r-1 rows with dx in
    {3-r, 3+r}, bottom row (2r+1). img_dims are AP dims [stride, num]
    prepended to [1, W]; img_off indexes the [7, 326] shifted-image block.
    """
    n = 2 * r + 1
    lo = 3 - r
    return [
        (0, n, [[1, n]], lo * PAD_W + lo),
        (n, 2 * (n - 2), [[PAD_W, n - 2], [2 * r, 2]], (lo + 1) * PAD_W + lo),
        (n + 2 * (n - 2), n, [[1, n]], (lo + n - 1) * PAD_W + lo),
    ]

_NC = None
LAST_RESULTS = None


def _build_program():
    f16 = mybir.dt.float16
    f32 = mybir.dt.float32
    nc = bacc.Bacc("TRN2", target_bir_lowering=False, debug=False, num_devices=8)
    k_d = nc.dram_tensor("k", [HALF_ROWS, KK, W], f16, kind="ExternalInput").ap()
    img7_d = nc.dram_tensor("img7", [HALF_ROWS, 7, PAD_W], f16, kind="ExternalInput").ap()
    in0_d = nc.dram_tensor("in0", [HALF_ROWS, W], f16, kind="ExternalInput").ap()
    att_d = nc.dram_tensor("att", [HALF_ROWS, 4, W], f16, kind="ExternalInput").ap()
    out_d = nc.dram_tensor("out", [HALF_ROWS, W], f32, kind="ExternalOutput").ap()

    with tile.TileContext(nc) as tc, ExitStack() as ctx:
        pool = ctx.enter_context(tc.tile_pool(name="main", bufs=1))

        # planes 0:49   ring-ordered k (DMA dst, read-only afterwards)
        # planes 49:98  products patches*k (plane 49+j for tap j)
        # planes 98:146 |k| (plane 97+j for tap j, j=1..48)
        kall = pool.tile([HALF_ROWS, 146, W], f16, name="kall")
        img7_t = pool.tile([HALF_ROWS, 7, PAD_W], f16)
        in0_t = pool.tile([HALF_ROWS, W], f16)
        att_t = pool.tile([HALF_ROWS, 4, W], f16)
        # planes 0:4 = B_r, planes 4:8 = A_r
        res = pool.tile([HALF_ROWS, 8, W], f16)

        kall_ap = kall[:]
        kpart = kall_ap.ap[0]
        img7_ap = img7_t[:]
        ipart = img7_ap.ap[0]

        def kap(plane, dims):
            return bass.AP(kall_ap.tensor, kall_ap.offset + plane * W,
                           [kpart] + dims)

        def iap(off, dims):
            return bass.AP(img7_ap.tensor, img7_ap.offset + off,
                           [ipart] + dims + [[1, W]])

        # ---- DMAs, ordered by when compute needs the data
        nc.sync.dma_start(kall[:, 1:5, :], k_d[:, 1:5, :])          # ring1
        nc.sync.dma_start(img7_t[:, 2, :], img7_d[:, 2, :])
        nc.sync.dma_start(in0_t[:], in0_d[:])
        nc.sync.dma_start(kall[:, 0:1, :], k_d[:, 0:1, :])          # center
        nc.sync.dma_start(kall[:, 5:9, :], k_d[:, 5:9, :])
        nc.sync.dma_start(img7_t[:, 3:5, :], img7_d[:, 3:5, :])     # ring1 rows
        nc.sync.dma_start(kall[:, 9:17, :], k_d[:, 9:17, :])        # ring2
        nc.sync.dma_start(kall[:, 17:25, :], k_d[:, 17:25, :])
        nc.sync.dma_start(img7_t[:, 1, :], img7_d[:, 1, :])
        nc.sync.dma_start(img7_t[:, 5, :], img7_d[:, 5, :])
        nc.sync.dma_start(kall[:, 25:33, :], k_d[:, 25:33, :])      # ring3
        nc.sync.dma_start(img7_t[:, 0, :], img7_d[:, 0, :])
        nc.sync.dma_start(img7_t[:, 6, :], img7_d[:, 6, :])
        nc.sync.dma_start(kall[:, 33:41, :], k_d[:, 33:41, :])
        nc.sync.dma_start(kall[:, 41:49, :], k_d[:, 41:49, :])
        nc.sync.dma_start(att_t[:], att_d[:])

        # ---- |k| on ScalarE; products go to separate planes, so this never
        # orders against the DVE multiplies
        Abs = mybir.ActivationFunctionType.Abs
        nc.scalar.activation(kall[:, 98:102, :], kall[:, 1:5, :], Abs)   # ring1
        nc.scalar.activation(kall[:, 102:106, :], kall[:, 5:9, :], Abs)
        nc.scalar.activation(res[:, 4, :], kall[:, 0, :], Abs)           # A_0
        for o in range(9, KK, 8):
            nc.scalar.activation(kall[:, o + ABASE:o + 8 + ABASE, :],
                                 kall[:, o:o + 8, :], Abs)

        # ---- patches*k multiplies (3 fused ops per ring + center)
        first = True
        for r in (1, 2, 3):
            for (rel, n_pl, img_dims, img_off) in _mul_ops(r):
                o = RING_OFF[r] + rel
                nc.vector.tensor_mul(kall[:, PBASE + o:PBASE + o + n_pl, :],
                                     kall[:, o:o + n_pl, :],
                                     iap(img_off, img_dims))
                if first:
                    # center tap fills the wait for the second ring1 chunk
                    nc.vector.tensor_mul(res[:, 0, :], kall[:, 0, :], in0_t[:])
                    first = False

        # ---- ring sums: each op handles the B level and the A level (48
        # planes up) through a paired leading AP dim
        def paired_fold(base, h, delta):
            """kall[{base, base+48}][0:h] += kall[{base+delta, ...}][0:h]"""
            dims = [[APLANE * W, 2], [1, h * W]]
            nc.vector.tensor_add(kap(base, dims), kap(base, dims),
                                 kap(base + delta, dims))

        def paired_tree(base, sz, r):
            cur = sz
            while cur > 2:
                paired_fold(base, cur // 2, cur // 2)
                cur //= 2
            dims = [[APLANE * W, 2], [1, W]]
            rdims = [[4 * W, 2], [1, W]]
            nc.vector.tensor_add(
                bass.AP(res[:].tensor, res[:].offset + r * W, [res[:].ap[0]] + rdims),
                kap(base, dims), kap(base + 1, dims))

        paired_tree(PBASE + 1, 8, 1)             # ring1
        paired_tree(PBASE + 9, 16, 2)            # ring2
        paired_fold(PBASE + 25, 8, 8)            # ring3: fold chunks
        paired_fold(PBASE + 25, 8, 16)
        paired_tree(PBASE + 25, 8, 3)

        # ---- combine + divide (att read twice: B planes, then A planes)
        pnd = pool.tile([HALF_ROWS, 8, W], f16)
        nc.vector.tensor_mul(pnd[:, 0:4, :], att_t[:], res[:, 0:4, :])
        nc.vector.tensor_mul(pnd[:, 4:8, :], att_t[:], res[:, 4:8, :])
        pnd_ap = pnd[:]
        ppart = pnd_ap.ap[0]

        def pap(plane, dims):
            return bass.AP(pnd_ap.tensor, pnd_ap.offset + plane * W,
                           [ppart] + dims)

        d2 = [[4 * W, 2], [1, 2 * W]]
        nc.vector.tensor_add(pap(0, d2), pap(0, d2), pap(2, d2))
        d1 = [[4 * W, 2], [1, W]]
        nc.vector.tensor_add(pap(0, d1), pap(0, d1), pap(1, d1))

        # divide in fp32: reciprocal_approx depends on fp32 bit layout
        den_t = pool.tile([HALF_ROWS, W], f32)
        nc.scalar.copy(den_t[:], pnd[:, 4, :])
        rden_t = pool.tile([HALF_ROWS, W], f32)
        scr_t = pool.tile([HALF_ROWS, W], f32)
        nc.vector.reciprocal_approx_accurate(rden_t[:], den_t[:], scr_t[:])
        out_t = pool.tile([HALF_ROWS, W], f32)
        nc.vector.tensor_mul(out_t[:], pnd[:, 0, :], rden_t[:])
        nc.sync.dma_start(out_d[:], out_t[:])

    nc.compile()
    return nc


def _get_program():
    global _NC
    if _NC is None:
        _NC = _build_program()
    return _NC


def kernel(**inputs) -> np.ndarray:
    k = np.asarray(inputs["kernel"], dtype=np.float32)      # [4, 49, 81920]
    img = np.asarray(inputs["input"], dtype=np.float32)     # [4, 1, 256, 320]
    in0 = np.asarray(inputs["input0"], dtype=np.float32)    # [4, 1, 256, 320]
    att = np.asarray(inputs["attention"], dtype=np.float32) # [4, 6, 4, 81920]
    ii = int(np.asarray(inputs["i"]))

    k16 = k.astype(np.float16)
    img16 = img.astype(np.float16)
    in016 = in0.astype(np.float16)
    att16 = att[:, ii].astype(np.float16)                   # [4, 4, 81920]

    nc = _get_program()

    in_maps = []
    for c in range(8):
        b, half = divmod(c, 2)
        y0 = half * HALF_ROWS
        kc = k16[b][RING_ORDER][:, y0 * W:(y0 + HALF_ROWS) * W]
        kc = np.ascontiguousarray(kc.reshape(KK, HALF_ROWS, W).transpose(1, 0, 2))
        pad = np.zeros((HALF_ROWS + 6, PAD_W), np.float16)
        lo, hi = max(0, y0 - 3), min(H, y0 + HALF_ROWS + 3)
        pad[lo - (y0 - 3):hi - (y0 - 3), 3:3 + W] = img16[b, 0, lo:hi]
        img7 = np.ascontiguousarray(
            np.stack([pad[t:t + HALF_ROWS] for t in range(7)], axis=1))
        in0c = np.ascontiguousarray(in016[b, 0, y0:y0 + HALF_ROWS])
        attc = att16[b][:, y0 * W:(y0 + HALF_ROWS) * W]
        attc = np.ascontiguousarray(
            attc.reshape(4, HALF_ROWS, W).transpose(1, 0, 2))
        in_maps.append({"k": kc, "img7": img7, "in0": in0c, "att": attc})

    res = run_bass_kernel_spmd(nc, in_maps, list(range(8)))
    global LAST_RESULTS
    LAST_RESULTS = res

    out = np.empty((BS, 1, H, W), np.float32)
    for c in range(8):
        b, half = divmod(c, 2)
        out[b, 0, half * HALF_ROWS:(half + 1) * HALF_ROWS] = res.results[c]["out"]
    return out
